# revision 18
# baseline (speedup 1.0000x reference)
"""Trainium2 Bass kernel for CrossAttention (B=4, N=2048, C=768, H=12).

Sharding: 8 cores = 4 head-groups (3 heads each) x 2 batch-groups (2 batches
each). Every core computes, for its (heads, batches):
    Q/K/V projections -> S^T = K @ Q^T -> exp(S^T) * exp(bias)^T (host
    precomputes EB = exp(bias); the multiply runs on the DVE in bf16 2x mode)
    -> PV (ones-augmented V gives softmax sums for free) -> normalize ->
    partial output projection.

Scheduling: the tensor-engine stream is kept dense so the PE holds its high
p-state. proj(b1) is emitted in chunks interleaved into the (h0,b0) attention
block, V(b1) chunks into (h0,b1), and oproj(b0) chunks into (h2,b1); PV
trails QK by TRAIL m-tiles so cross-engine waits are pre-satisfied. EB tiles
stream one head ahead (h0 during the proj phase, h+1 during (h,b1)).
O overwrites Q in-place (head-sliced), saving SBUF. Host sums the 4
head-group partial outputs and adds the projection bias.
"""

import sys

for _p in ("/opt/trn_rl_repo",):
    if _p not in sys.path:
        sys.path.insert(0, _p)

import numpy as np
import ml_dtypes

B, N, C, H, D = 4, 2048, 768, 12, 64
SCALE = D ** -0.5
HG, BG = 4, 2            # head-groups x batch-groups = 8 cores
HL = H // HG             # 3 heads per core
BL = B // BG             # 2 batches per core
MT = N // 128            # 16 m tiles (key tiles)
CT = C // 128            # 6 c tiles
TRAIL = 3                # PV trails QK by this many m-tiles
BF16 = ml_dtypes.bfloat16

_prog_cache = {}


def _build_program():
    import concourse.bass as bass
    import concourse.tile as tile
    from concourse import bacc, mybir

    f32 = mybir.dt.float32
    bf16 = mybir.dt.bfloat16

    nc = bacc.Bacc("TRN2", target_bir_lowering=False, debug=False)

    xT = nc.dram_tensor("xT", [BL, C, N], bf16, kind="ExternalInput")
    kT = nc.dram_tensor("kT", [BL, C, N], bf16, kind="ExternalInput")
    vT = nc.dram_tensor("vT", [BL, C, N], bf16, kind="ExternalInput")
    ebT = nc.dram_tensor("ebT", [HL, N, N], bf16, kind="ExternalInput")
    wq = nc.dram_tensor("wq", [C, HL * D], bf16, kind="ExternalInput")
    wk = nc.dram_tensor("wk", [C, HL * D], bf16, kind="ExternalInput")
    wv = nc.dram_tensor("wv", [C, HL * D], bf16, kind="ExternalInput")
    wp = nc.dram_tensor("wp", [HL * D, C], bf16, kind="ExternalInput")
    yT = nc.dram_tensor("yT", [BL, C, N], bf16, kind="ExternalOutput")

    GD = HL * D  # 192

    with tile.TileContext(nc) as tc:
        with (
            tc.tile_pool(name="wpool", bufs=1) as wpool,
            tc.tile_pool(name="bigp", bufs=12) as bigp,
            tc.tile_pool(name="ebp", bufs=16) as ebp,
            tc.tile_pool(name="persist", bufs=1) as persist,
            tc.tile_pool(name="ppool", bufs=1) as ppool,
            tc.tile_pool(name="miscp", bufs=1) as miscp,
            tc.tile_pool(name="ypool", bufs=2) as ypool,
            tc.tile_pool(name="ps", bufs=2, space="PSUM") as ps,
            tc.tile_pool(name="po", bufs=1, space="PSUM") as po,
        ):
            # ---- weights ----
            wq_sb = wpool.tile([128, CT * GD], bf16, tag="wq")
            nc.sync.dma_start(wq_sb.rearrange("p (t d) -> p t d", d=GD),
                              wq.rearrange("(t p) d -> p t d", p=128))
            wk_sb = wpool.tile([128, CT * GD], bf16, tag="wk")
            nc.sync.dma_start(wk_sb.rearrange("p (t d) -> p t d", d=GD),
                              wk.rearrange("(t p) d -> p t d", p=128))
            wv_sb = wpool.tile([128, CT * GD], bf16, tag="wv")
            nc.sync.dma_start(wv_sb.rearrange("p (t d) -> p t d", d=GD),
                              wv.rearrange("(t p) d -> p t d", p=128))
            wp0_sb = wpool.tile([128, C], bf16, tag="wp0")
            nc.sync.dma_start(wp0_sb[:], wp[0:128, :])
            # wp1 duplicated on both partition halves so its base partition
            # matches on2's (head-2 tiles pack b0 at 0-63, b1 at 64-127)
            wp1_sb = wpool.tile([128, C], bf16, tag="wp1")
            nc.sync.dma_start(wp1_sb[0:64, :], wp[128:192, :])
            nc.sync.dma_start(wp1_sb[64:128, :], wp[128:192, :])

            # head groups: (Q/K partition offset, size); heads 0,1 packed
            groups = [(0, 128), (128, 64)]

            # ---- persistent per-batch tensors ----
            # qT01/qT2 hold Q before attention; the normalized O overwrites
            # them head-slice by head-slice at each block's norm (the slice's
            # last Q read precedes its O write in program order).
            qT01, qT2, kT01, kT2 = {}, {}, {}, {}
            vaug = {}
            q2pack = persist.tile([128, N], bf16, tag="q2p", name="q2p")
            k2pack = persist.tile([128, N], bf16, tag="k2p", name="k2p")
            for b in range(BL):
                qT01[b] = persist.tile([128, N], bf16, tag=f"q01_{b}", name=f"q01_{b}")
                qT2[b] = q2pack[b * 64:(b + 1) * 64, :]
                kT01[b] = persist.tile([128, N], bf16, tag=f"k01_{b}", name=f"k01_{b}")
                kT2[b] = k2pack[b * 64:(b + 1) * 64, :]
                # V layout: [128 tokens, (mt, h, D+1)], ones column at D
                vaug[b] = persist.tile([128, MT * HL * (D + 1)], bf16,
                                       tag=f"v_{b}", name=f"v_{b}")
                va4 = vaug[b].rearrange("p (t h c) -> p t h c", h=HL, c=D + 1)
                nc.gpsimd.memset(va4[:, :, :, D], 1.0)

            # ================= projections =================
            def emit_input_loads(b):
                """Allocate + DMA the 18 input tiles for batch b."""
                tiles = {}
                for nm, src in (("x", xT), ("k", kT), ("v", vT)):
                    for ct in range(CT):
                        t = bigp.tile([128, N], bf16, tag="big", name="big_t")
                        nc.gpsimd.dma_start(t[:], src[b, ct * 128:(ct + 1) * 128, :])
                        tiles[(nm, ct)] = t
                return tiles

            def proj_chunks(b, tiles, cast_engine):
                """32 chunks: Q (8), K (8), V (16). cast_engine does the
                psum->sbuf copies."""
                chunks = []
                for nm, w_sb, d01, d2 in (("x", wq_sb, qT01[b], qT2[b]),
                                          ("k", wk_sb, kT01[b], kT2[b])):
                    for goff, gsz in groups:
                        for nb in range(2):
                            def qk_chunk(nm=nm, w_sb=w_sb, d01=d01, d2=d2,
                                         goff=goff, gsz=gsz, nb=nb):
                                pq = ps.tile([gsz, 1024], f32, tag="s", name="pq")
                                for half in range(2):
                                    off = nb * 1024 + half * 512
                                    for ct in range(CT):
                                        nc.tensor.matmul(
                                            pq[:, half * 512:(half + 1) * 512],
                                            w_sb[:, ct * GD + goff:
                                                 ct * GD + goff + gsz],
                                            tiles[(nm, ct)][:, off:off + 512],
                                            start=(ct == 0), stop=(ct == CT - 1))
                                dst = d01 if gsz == 128 else d2
                                cast_engine.tensor_copy(
                                    dst[:, nb * 1024:(nb + 1) * 1024], pq[:])
                            chunks.append(qk_chunk)
                va4 = vaug[b].rearrange("p (t h c) -> p t h c", h=HL, c=D + 1)
                for mt in range(MT):
                    def v_chunk(mt=mt):
                        pv = ps.tile([128, GD], f32, tag="s", name="pv")
                        for ct in range(CT):
                            nc.tensor.matmul(
                                pv[:], tiles[("v", ct)][:, mt * 128:(mt + 1) * 128],
                                wv_sb[:, ct * GD:(ct + 1) * GD],
                                start=(ct == 0), stop=(ct == CT - 1))
                        cast_engine.tensor_copy(
                            va4[:, mt, :, 0:D],
                            pv.rearrange("p (h d) -> p h d", d=D))
                    chunks.append(v_chunk)
                return chunks

            # ================= output projection =================
            def oproj_chunks(b):
                """12 chunks of [128, 1024] psum each; Pool does the casts."""
                on01 = qT01[b]
                on2 = qT2[b]
                chunks = []
                for ct in range(CT):
                    for nb2 in range(2):
                        def o_chunk(ct=ct, nb2=nb2):
                            y_sb = ypool.tile([128, 1024], bf16, tag="y",
                                              name="y_t")
                            py = ps.tile([128, 1024], f32, tag="s", name="py")
                            for hf in range(2):
                                sl = slice(nb2 * 1024 + hf * 512,
                                           nb2 * 1024 + (hf + 1) * 512)
                                nc.tensor.matmul(
                                    py[:, hf * 512:(hf + 1) * 512],
                                    wp0_sb[:, ct * 128:(ct + 1) * 128],
                                    on01[:, sl], start=True, stop=False)
                                nc.tensor.matmul(
                                    py[:, hf * 512:(hf + 1) * 512],
                                    wp1_sb[b * 64:(b + 1) * 64,
                                           ct * 128:(ct + 1) * 128],
                                    on2[:, sl], start=False, stop=True)
                            nc.vector.tensor_copy(y_sb[:], py[:])
                            nc.gpsimd.dma_start(
                                yT[b, ct * 128:(ct + 1) * 128,
                                   nb2 * 1024:(nb2 + 1) * 1024], y_sb[:])
                        chunks.append(o_chunk)
                return chunks

            # ================= attention =================
            ebtiles = {}

            def load_eb(h, mt):
                t = ebp.tile([128, N], bf16, tag="eb", name="eb_t")
                nc.sync.dma_start(t[:], ebT[h, mt * 128:(mt + 1) * 128, :])
                ebtiles[(h, mt)] = t

            def attention_block(h, b, extra=()):
                if h < 2:
                    k_src = kT01[b][h * D:(h + 1) * D, :]
                    q_src = qT01[b][h * D:(h + 1) * D, :]
                else:
                    k_src = kT2[b][:, :]
                    q_src = qT2[b][:, :]
                va4 = vaug[b].rearrange("p (t h c) -> p t h c", h=HL, c=D + 1)

                pos = po.tile([D + 1, N], f32, tag="o", name="pos")
                pts = {}
                n_iter = MT + TRAIL
                ci = 0
                for mt in range(n_iter):
                    if mt < MT:
                        ebt = ebtiles[(h, mt)]
                        pt = ppool.tile([128, N], bf16, tag="pe",
                                        bufs=TRAIL + 2, name="p_e")
                        for half in range(2):
                            sp = ps.tile([128, 1024], f32, tag="s", name="sp")
                            for hf in range(2):
                                off = half * 1024 + hf * 512
                                nc.tensor.matmul(
                                    sp[:, hf * 512:(hf + 1) * 512],
                                    k_src[:, mt * 128:(mt + 1) * 128],
                                    q_src[:, off:off + 512],
                                    start=True, stop=True)
                            nc.scalar.activation(
                                pt[:, half * 1024:(half + 1) * 1024], sp[:],
                                mybir.ActivationFunctionType.Exp)
                        # EB multiply in place (bf16 2x mode on the DVE)
                        nc.vector.tensor_mul(pt[:], pt[:], ebt[:])
                        pts[mt] = pt
                        # EB prefetch, emitted after this m-tile's consumers
                        # so slot-reuse dependencies are complete:
                        # h0 streams JIT within its own b0 block; later heads
                        # stream during the previous head's b1 block.
                        if h == 0 and b == 0 and mt + 2 < MT:
                            load_eb(0, mt + 2)
                        if b == 1 and h + 1 < HL:
                            load_eb(h + 1, mt)
                    if TRAIL <= mt:
                        pm = mt - TRAIL
                        vsl = va4[:, pm, h, :]
                        pt2 = pts.pop(pm)
                        for q4 in range(4):
                            nc.tensor.matmul(
                                pos[:, q4 * 512:(q4 + 1) * 512], vsl,
                                pt2[:, q4 * 512:(q4 + 1) * 512],
                                start=(pm == 0), stop=(pm == MT - 1))
                    # pace the extra tensor work to finish by iteration MT-1
                    # (interleaved V(b1) chunks must precede their PV reads)
                    want = ((mt + 1) * len(extra) + MT - 1) // MT
                    while ci < min(want, len(extra)):
                        extra[ci]()
                        ci += 1

                # normalization: rows 0..63 of pos are O^T, row 64 the sums;
                # O overwrites this head's Q slice.
                sum_sb = miscp.tile([1, N], f32, tag="sum_sb", name="sum_sb")
                rec_f = miscp.tile([1, N], f32, tag="rec_f", name="rec_f")
                rb = miscp.tile([D, N], f32, tag="rb", name="rb")
                # custom-DVE reciprocal can't read PSUM: stage sums to SBUF
                nc.vector.tensor_copy(sum_sb[:], pos[D:D + 1, :])
                nc.vector.reciprocal_approx_fast(rec_f[:], sum_sb[:])
                nc.gpsimd.partition_broadcast(rb[:], rec_f[:])
                nc.vector.tensor_mul(q_src, pos[0:D, :], rb[:])

            # ================= schedule =================
            tiles_b0 = emit_input_loads(0)
            for ch in proj_chunks(0, tiles_b0, nc.vector):
                ch()
            load_eb(0, 0)
            load_eb(0, 1)

            tiles_b1 = emit_input_loads(1)
            b1_chunks = proj_chunks(1, tiles_b1, nc.vector)
            attention_block(0, 0, extra=b1_chunks[:16])   # Q/K of b1
            attention_block(0, 1, extra=b1_chunks[16:])   # V of b1
            attention_block(1, 0)
            attention_block(1, 1)
            attention_block(2, 0)
            attention_block(2, 1, extra=oproj_chunks(0))
            for ch in oproj_chunks(1):
                ch()
    nc.compile()
    return nc


def get_program():
    key = "nc"
    if key not in _prog_cache:
        _prog_cache[key] = _build_program()
    return _prog_cache[key]


def make_in_maps(x, k_in, v_in, rel_pos_bias, Wq, Wk, Wv, Wp):
    xT = x.transpose(0, 2, 1).astype(BF16)
    kT = k_in.transpose(0, 2, 1).astype(BF16)
    vT = v_in.transpose(0, 2, 1).astype(BF16)
    ebT = np.exp(rel_pos_bias.transpose(0, 2, 1)).astype(BF16)  # [H, m, n]
    WqT = (Wq * SCALE).T.astype(BF16)                       # [C, C]
    WkT = Wk.T.astype(BF16)
    WvT = Wv.T.astype(BF16)
    WpT = Wp.T.astype(BF16)                                 # [C(d_in), C]

    in_maps = []
    for c in range(8):
        hg, bg = c % HG, c // HG
        hs, bs = hg * HL, bg * BL
        in_maps.append({
            "xT": np.ascontiguousarray(xT[bs:bs + BL]),
            "kT": np.ascontiguousarray(kT[bs:bs + BL]),
            "vT": np.ascontiguousarray(vT[bs:bs + BL]),
            "ebT": np.ascontiguousarray(ebT[hs:hs + HL]),
            "wq": np.ascontiguousarray(WqT[:, hs * D:(hs + HL) * D]),
            "wk": np.ascontiguousarray(WkT[:, hs * D:(hs + HL) * D]),
            "wv": np.ascontiguousarray(WvT[:, hs * D:(hs + HL) * D]),
            "wp": np.ascontiguousarray(WpT[hs * D:(hs + HL) * D, :]),
        })
    return in_maps


def assemble_output(results, bp):
    y = np.zeros((B, C, N), dtype=np.float32)
    for c in range(8):
        hg, bg = c % HG, c // HG
        bs = bg * BL
        y[bs:bs + BL] += results[c]["yT"].astype(np.float32)
    out = y.transpose(0, 2, 1) + bp.astype(np.float32)
    return np.ascontiguousarray(out.astype(np.float32))


def kernel(**inputs):
    from concourse.bass_utils import run_bass_kernel_spmd

    x = np.asarray(inputs["x"], dtype=np.float32)
    k_in = np.asarray(inputs["k_in"], dtype=np.float32)
    v_in = np.asarray(inputs["v_in"], dtype=np.float32)
    rel_pos_bias = np.asarray(inputs["rel_pos_bias"], dtype=np.float32)
    Wq = np.asarray(inputs["Wq"], dtype=np.float32)
    Wk = np.asarray(inputs["Wk"], dtype=np.float32)
    Wv = np.asarray(inputs["Wv"], dtype=np.float32)
    Wp = np.asarray(inputs["Wp"], dtype=np.float32)
    bp = np.asarray(inputs["bp"], dtype=np.float32)

    nc = get_program()
    in_maps = make_in_maps(x, k_in, v_in, rel_pos_bias, Wq, Wk, Wv, Wp)
    res = run_bass_kernel_spmd(nc, in_maps, list(range(8)))
    return assemble_output(res.results, bp)
